# revision 5
# baseline (speedup 1.0000x reference)
"""Fused multi-head attention (2 heads, RoPE-across-heads) on 8 Trainium2 NeuronCores.

Reference computation (per batch b of 4, seq 2048, dim 2048):
    qkv = x @ wqkv; rope mixes the two heads; scores = q'k'^T/32; softmax;
    out = (attn @ v) @ wout + bout

Sharding: core c owns (batch = c//2, seq-half = c%2) -> 1024 query rows.
Each core projects q/k/v for its own 1024 rows, ropes q/k, AllGathers
k'/v within the (2c, 2c+1) pair, runs attention for its rows against the
full 2048-seq k'/v, and applies the output projection for its rows.

On-device layouts (partition dim first):
    xT    [dim, rows]      - rhs/stationary for projections
    q'T   [2048, rows]     - head-dim on partitions (chunked [128,16,1024])
    k'T   [2048, rows]     - gathered to k_g [2*2048, 1024] (stacked shards)
    v     [rows, 2048]     - natural; gathered to v_g [2048, 2048]
    P^T   [seq_j, rows]    - exp(scores^T), bf16
    aoT   [2048, rows]     - unnormalized attn-out^T, normalized on write

Softmax skips max-subtraction: scores = q'.k'/32 ~ N(0,1), |scores| < ~8,
so exp is safe in f32 (verified against the reference distribution).

Scheduling notes (v2): weight-load DMA triggers are kept on queues with no
dependency-stalled compute ahead of them so prefetch actually runs ahead;
k'T for attention is streamed in small [128,8,128] tiles from a persistent
pool so the loads overlap the q projection; per head, both row-blocks'
scores run before any attn@v so the v AllGather has extra slack; the first
wout tile is prefetched during attention via a pool opened early.
"""

import os
import sys

import numpy as np

if "/opt/trn_rl_repo" not in sys.path:
    sys.path.insert(0, "/opt/trn_rl_repo")

import ml_dtypes

# ---------------------------------------------------------------- constants
B, S, D = 4, 2048, 2048          # batch, seq, model dim
H, HD = 2, 1024                  # heads, head dim
R = 1024                         # query rows per core
N_CORES = 8
SCALE = 1.0 / 32.0               # HD ** -0.5
NRB = R // 512                   # 512-row blocks

_NC_CACHE = {}
LAST_RESULT = {}


def _build():
    import concourse.bass as bass
    import concourse.tile as tile
    from concourse import bacc, mybir

    F32 = mybir.dt.float32
    F16 = mybir.dt.float16
    BF = mybir.dt.bfloat16
    Exp = mybir.ActivationFunctionType.Exp

    nc = bacc.Bacc("TRN2", target_bir_lowering=False, debug=False,
                   num_devices=N_CORES)

    xT = nc.dram_tensor("xT", [D, R], BF, kind="ExternalInput").ap()
    wqkv = nc.dram_tensor("wqkv", [D, 3 * D], BF, kind="ExternalInput").ap()
    wout = nc.dram_tensor("wout", [D, D], BF, kind="ExternalInput").ap()
    cosk = nc.dram_tensor("cosk", [512, R], F16, kind="ExternalInput").ap()
    sink = nc.dram_tensor("sink", [512, R], F16, kind="ExternalInput").ap()
    bias = nc.dram_tensor("bias", [1, D], F32, kind="ExternalInput").ap()
    out = nc.dram_tensor("out", [R, D], F32, kind="ExternalOutput").ap()

    wq_r = wqkv.rearrange("(c p) m -> p c m", p=128)    # [128, 16, 6144]
    x_r = xT.rearrange("(c p) r -> p c r", p=128)       # [128, 16, R]
    ck_r = cosk.rearrange("(c p) r -> p c r", p=128)    # [128, 4, R]
    sk_r = sink.rearrange("(c p) r -> p c r", p=128)

    def bcast_ap(src_ap, nparts, width):
        return bass.AP(tensor=src_ap.tensor, offset=src_ap.offset,
                       ap=[[0, nparts], [1, width]])

    with tile.TileContext(nc) as tc:
        with (
            tc.tile_pool(name="persist", bufs=1) as persist,
            tc.tile_pool(name="psum", bufs=6, space="PSUM") as psp,
            tc.tile_pool(name="dram", bufs=1, space="DRAM") as dram,
        ):
            # ------------------------------------------- persistent buffers
            qT_sb = persist.tile([128, 16, R], BF, tag="qT")
            aoT_sb = persist.tile([128, 16, R], BF, tag="aoT")
            bias_sb = persist.tile([128, D], F32, tag="bias")
            ones_sb = persist.tile([128, 1], BF, tag="ones")
            nc.vector.memset(ones_sb, 1.0)

            # DRAM scratch
            k_in = dram.tile([D, R], BF, tag="k_in")
            v_in = dram.tile([R, D], BF, tag="v_in")
            k_g = dram.tile([2 * D, R], BF, tag="k_g")     # stacked k'T shards
            v_g = dram.tile([S, D], BF, tag="v_g")         # natural v, global rows

            # =================================================== projections
            with (
                tc.tile_pool(name="proj", bufs=1) as proj,
                tc.tile_pool(name="projs", bufs=1) as projs,
            ):
                # x: interleave the 16 column-chunk loads across two queues so
                # the first accumulation chains are fed at 2x ring bandwidth
                x_sb = proj.tile([128, 16, R], BF, tag="xsb")
                for kc in range(16):
                    eng = nc.sync if kc % 2 == 0 else nc.scalar
                    eng.dma_start(out=x_sb[:, kc, :], in_=x_r[:, kc, :])
                # first stationary weights for the k projection
                w_first = []
                for c in (0, 8):
                    wt = projs.tile([128, 16, 128], BF, tag="wst", bufs=3)
                    for hh in (0, 8):
                        nc.gpsimd.dma_start(
                            out=wt[:, hh:hh + 8, :],
                            in_=wq_r[:, hh:hh + 8, D + c * 128:D + (c + 1) * 128])
                    w_first.append(wt)
                # rope tables; split across queues, needed ~8us in
                ck_sb = proj.tile([128, 4, R], F16, tag="ck")
                sk_sb = proj.tile([128, 4, R], F16, tag="sk")
                nc.gpsimd.dma_start(out=ck_sb, in_=ck_r)
                nc.scalar.dma_start(out=sk_sb, in_=sk_r)
                nc.gpsimd.dma_start(out=bias_sb, in_=bcast_ap(bias, 128, D))
                # hoist the first v-projection weight block (consumed only
                # after the whole k projection; sync queue never blocks)
                wv0 = projs.tile([128, 16, 512], BF, tag="wv", bufs=2)
                for kc in range(0, 16, 4):
                    nc.sync.dma_start(
                        out=wv0[:, kc:kc + 4, :],
                        in_=wq_r[:, kc:kc + 4, 2 * D:2 * D + 512])

                def load_wst(col0, cc0, dma_eng):
                    wt = projs.tile([128, 16, 128], BF, tag="wst", bufs=3)
                    for hh in (0, 8):
                        dma_eng.dma_start(
                            out=wt[:, hh:hh + 8, :],
                            in_=wq_r[:, hh:hh + 8,
                                     col0 + cc0 * 128:col0 + (cc0 + 1) * 128])
                    return wt

                def qk_proj(col0, dest, dma_eng, preloaded=None, post=None):
                    """Project+rope cols [col0, col0+2048) of wqkv.

                    dest(c, rb) -> (apA, apB): bf16 [128,512] destination APs
                    for col-chunk c (head0) and c+8 (head1), row block rb.
                    post(c, rb, apA, apB), if given, runs after the writes."""
                    for c in range(8):
                        if c == 0 and preloaded is not None:
                            w1, w2 = preloaded
                        else:
                            w1 = load_wst(col0, c, dma_eng)
                            w2 = load_wst(col0, c + 8, dma_eng)
                        for rb in range(NRB):
                            rs = slice(rb * 512, (rb + 1) * 512)
                            ps1 = psp.tile([128, 512], F32, tag="mm")
                            ps2 = psp.tile([128, 512], F32, tag="mm")
                            for kc in range(16):
                                nc.tensor.matmul(ps1, w1[:, kc, :], x_sb[:, kc, rs],
                                                 start=kc == 0, stop=kc == 15)
                            for kc in range(16):
                                nc.tensor.matmul(ps2, w2[:, kc, :], x_sb[:, kc, rs],
                                                 start=kc == 0, stop=kc == 15)
                            cosv = ck_sb[:, c % 4, rs]
                            sinv = sk_sb[:, c % 4, rs]
                            t1 = projs.tile([128, 512], F32, tag="rt", bufs=4)
                            t2 = projs.tile([128, 512], F32, tag="rt", bufs=4)
                            apA, apB = dest(c, rb)
                            nc.vector.tensor_mul(t1, ps1, cosv)
                            nc.vector.tensor_mul(t2, ps2, sinv)
                            nc.vector.tensor_sub(apA, t1, t2)
                            nc.vector.tensor_mul(t1, ps2, cosv)
                            nc.vector.tensor_mul(t2, ps1, sinv)
                            nc.vector.tensor_add(apB, t1, t2)
                            if post is not None:
                                post(c, rb, apA, apB)

                # ---- k projection + rope -> k_in, AllGather within pair
                def dest_k(c, rb):
                    oA = projs.tile([128, 512], BF, tag="ro", bufs=4)
                    oB = projs.tile([128, 512], BF, tag="ro", bufs=4)
                    return oA, oB

                def post_k(c, rb, oA, oB):
                    rs = slice(rb * 512, (rb + 1) * 512)
                    nc.gpsimd.dma_start(out=k_in[c * 128:(c + 1) * 128, rs], in_=oA)
                    nc.gpsimd.dma_start(out=k_in[(c + 8) * 128:(c + 9) * 128, rs], in_=oB)

                qk_proj(D, dest_k, nc.scalar, preloaded=w_first, post=post_k)
                nc.gpsimd.collective_compute(
                    "AllGather", bass.mybir.AluOpType.bypass,
                    replica_groups=[[0, 1], [2, 3], [4, 5], [6, 7]],
                    ins=[k_in.opt()], outs=[k_g.opt()])

                # ---- v projection (natural layout); wv loads ride the sync
                # queue so the next block prefetches while this one computes
                for vc in range(4):
                    if vc == 0:
                        wv = wv0
                    else:
                        wv = projs.tile([128, 16, 512], BF, tag="wv", bufs=2)
                        for kc in range(0, 16, 4):
                            nc.sync.dma_start(
                                out=wv[:, kc:kc + 4, :],
                                in_=wq_r[:, kc:kc + 4, 2 * D + vc * 512:2 * D + (vc + 1) * 512])
                    for rr in range(R // 128):
                        ps = psp.tile([128, 512], F32, tag="mm")
                        for kc in range(16):
                            nc.tensor.matmul(ps, x_sb[:, kc, rr * 128:(rr + 1) * 128],
                                             wv[:, kc, :], start=kc == 0, stop=kc == 15)
                        vt = projs.tile([128, 512], BF, tag="vo", bufs=4)
                        nc.scalar.copy(vt, ps)
                        nc.scalar.dma_start(
                            out=v_in[rr * 128:(rr + 1) * 128, vc * 512:(vc + 1) * 512],
                            in_=vt)
                nc.gpsimd.collective_compute(
                    "AllGather", bass.mybir.AluOpType.bypass,
                    replica_groups=[[0, 1], [2, 3], [4, 5], [6, 7]],
                    ins=[v_in.opt()], outs=[v_g.opt()])

                # ---- q projection + rope, written directly into qT_sb
                def dest_q(c, rb):
                    rs = slice(rb * 512, (rb + 1) * 512)
                    return qT_sb[:, c, rs], qT_sb[:, c + 8, rs]

                qk_proj(0, dest_q, nc.sync)

            # ===================================================== attention
            # fin pool opens first so the wout prefetch below lives through
            # the attention phase
            with tc.tile_pool(name="fin", bufs=1) as fin:
                wout_r = wout.rearrange("(c p) m -> p c m", p=128)
                wo0 = fin.tile([128, 16, 512], BF, tag="wo", bufs=2)
                for dc in range(0, 16, 2):
                    nc.scalar.dma_start(out=wo0[:, dc:dc + 2, :],
                                        in_=wout_r[:, dc:dc + 2, 0:512])

                with tc.tile_pool(name="attn", bufs=1) as attn:
                    for hi in range(H):
                        v_sb = attn.tile([128, 16, HD], BF, tag="vh")
                        for jc in range(0, 16, 4):
                            nc.gpsimd.dma_start(
                                out=v_sb[:, jc:jc + 4, :],
                                in_=v_g[jc * 128:(jc + 4) * 128,
                                        hi * HD:(hi + 1) * HD].rearrange(
                                    "(c p) m -> p c m", p=128))
                        PTs, recs = [], []
                        # both row blocks' scores first: attn@v then has
                        # ~55us of slack on the v AllGather
                        for rb in range(NRB):
                            rs = slice(rb * 512, (rb + 1) * 512)
                            PT = attn.tile([128, 16, 512], BF, tag="PT", bufs=2)
                            for jc in range(16):
                                sh, jcc = jc // 8, jc % 8
                                base = sh * D + hi * HD
                                kts = persist.tile([128, 8, 128], BF,
                                                   tag="kts", bufs=4)
                                nc.sync.dma_start(
                                    out=kts,
                                    in_=k_g[base:base + 1024,
                                            jcc * 128:(jcc + 1) * 128].rearrange(
                                        "(c p) r -> p c r", p=128))
                                ps = psp.tile([128, 512], F32, tag="mm")
                                for dc in range(8):
                                    nc.tensor.matmul(
                                        ps, kts[:, dc, :],
                                        qT_sb[:, hi * 8 + dc, rs],
                                        start=dc == 0, stop=dc == 7)
                                nc.scalar.activation(PT[:, jc, :], ps, Exp, scale=SCALE)
                            # row sums via ones-matmul, reciprocal, DMA-bcast
                            sps = psp.tile([1, 512], F32, tag="sum", bufs=2)
                            for jc in range(16):
                                nc.tensor.matmul(sps, ones_sb, PT[:, jc, :],
                                                 start=jc == 0, stop=jc == 15)
                            rec = attn.tile([1, 512], F32, tag="rec", bufs=2)
                            nc.vector.reciprocal(rec, sps)
                            rec_d = dram.tile([1, 512], F32, tag="rec_d", bufs=2)
                            nc.scalar.dma_start(out=rec_d, in_=rec)
                            rec_b = attn.tile([128, 512], F32, tag="rec_b", bufs=2)
                            nc.scalar.dma_start(out=rec_b, in_=bcast_ap(rec_d, 128, 512))
                            PTs.append(PT)
                            recs.append(rec_b)
                        for rb in range(NRB):
                            rs = slice(rb * 512, (rb + 1) * 512)
                            PT, rec_b = PTs[rb], recs[rb]
                            for m in range(8):
                                pa = psp.tile([128, 512], F32, tag="mm")
                                for jc in range(16):
                                    nc.tensor.matmul(
                                        pa, v_sb[:, jc, m * 128:(m + 1) * 128],
                                        PT[:, jc, :], start=jc == 0, stop=jc == 15)
                                nc.vector.tensor_mul(aoT_sb[:, hi * 8 + m, rs], pa, rec_b)

                # ============================================ output projection
                for cc in range(4):
                    if cc == 0:
                        wo = wo0
                    else:
                        wo = fin.tile([128, 16, 512], BF, tag="wo", bufs=2)
                        for dc in range(0, 16, 2):
                            nc.scalar.dma_start(
                                out=wo[:, dc:dc + 2, :],
                                in_=wout_r[:, dc:dc + 2, cc * 512:(cc + 1) * 512])
                    for rr in range(R // 128):
                        r0 = rr * 128
                        ps = psp.tile([128, 512], F32, tag="mm")
                        for dc in range(16):
                            nc.tensor.matmul(ps, aoT_sb[:, dc, r0:r0 + 128],
                                             wo[:, dc, :],
                                             start=dc == 0, stop=dc == 15)
                        ot = fin.tile([128, 512], F32, tag="ot", bufs=4)
                        nc.vector.tensor_add(ot, ps, bias_sb[:, cc * 512:(cc + 1) * 512])
                        nc.gpsimd.dma_start(
                            out=out[r0:r0 + 128, cc * 512:(cc + 1) * 512], in_=ot)

    nc.compile()
    return nc


def _get_nc():
    if "v2" not in _NC_CACHE:
        _NC_CACHE["v2"] = _build()
    return _NC_CACHE["v2"]


def _rope_tables():
    inv_freq = 1.0 / (10000.0 ** (np.arange(0, HD, 2, dtype=np.float32) / HD))
    t = np.arange(S, dtype=np.float32)
    freqs = t[:, None] * inv_freq[None, :]          # (S, 512)
    return np.cos(freqs).astype(np.float32), np.sin(freqs).astype(np.float32)


def kernel(x, wqkv, wout, bout):
    from concourse.bass_utils import run_bass_kernel_spmd

    bf16 = ml_dtypes.bfloat16
    x = np.asarray(x, dtype=np.float32)
    wqkv_b = np.ascontiguousarray(np.asarray(wqkv, dtype=np.float32)).astype(bf16)
    wout_b = np.ascontiguousarray(np.asarray(wout, dtype=np.float32)).astype(bf16)
    bout_f = np.asarray(bout, dtype=np.float32).reshape(1, D)
    cos_h, sin_h = _rope_tables()                   # (S, 512) f32
    cosT = np.ascontiguousarray(cos_h.T)            # (512, S)
    sinT = np.ascontiguousarray(sin_h.T)

    nc = _get_nc()

    in_maps = []
    for c in range(N_CORES):
        bi, half = c // 2, c % 2
        rows = slice(half * R, (half + 1) * R)
        m = {
            "xT": np.ascontiguousarray(x[bi, rows, :].T).astype(bf16),
            "wqkv": wqkv_b,
            "wout": wout_b,
            "bias": bout_f,
            "cosk": np.ascontiguousarray(cosT[:, rows]).astype(np.float16),
            "sink": np.ascontiguousarray(sinT[:, rows]).astype(np.float16),
        }
        in_maps.append(m)

    trace = os.environ.get("KERNEL_TRACE", "0") == "1"
    res = run_bass_kernel_spmd(nc, in_maps, list(range(N_CORES)), trace=trace)
    if trace:
        LAST_RESULT["exec_time_ns"] = res.exec_time_ns
        LAST_RESULT["mean_exec_time_ns"] = res.mean_exec_time_ns

    out_full = np.empty((B, S, D), np.float32)
    for c in range(N_CORES):
        bi, half = c // 2, c % 2
        out_full[bi, half * R:(half + 1) * R, :] = res.results[c]["out"]
    return out_full


# revision 7
# speedup vs baseline: 1.0395x; 1.0395x over previous
"""Fused multi-head attention (2 heads, RoPE-across-heads) on 8 Trainium2 NeuronCores.

Reference computation (per batch b of 4, seq 2048, dim 2048):
    qkv = x @ wqkv; rope mixes the two heads; scores = q'k'^T/32; softmax;
    out = (attn @ v) @ wout + bout

Sharding: core c owns (batch = c//2, seq-half = c%2) -> 1024 query rows.
Each core projects q/k/v for its own 1024 rows, ropes q/k, AllGathers
k'/v within the (2c, 2c+1) pair, runs attention for its rows against the
full 2048-seq k'/v, and applies the output projection for its rows.

On-device layouts (partition dim first):
    xT    [dim, rows]      - rhs/stationary for projections
    q'T   [2048, rows]     - head-dim on partitions (chunked [128,16,1024])
    k'T   [2048, rows]     - gathered to k_g [2*2048, 1024] (stacked shards)
    v     [rows, 2048]     - natural; gathered to v_g [2048, 2048]
    P^T   [seq_j, rows]    - exp(scores^T), bf16
    aoT   [2048, rows]     - unnormalized attn-out^T, normalized on write

Softmax skips max-subtraction: scores = q'.k'/32 ~ N(0,1), |scores| < ~8,
so exp is safe in f32 (verified against the reference distribution).

Scheduling (v3), from perfetto evidence:
  - the sync DMA ring starves almost completely while an AllGather is in
    flight, so every load that must land inside an AG window rides the
    scalar ring (which keeps moving) and everything on sync is scheduled
    to complete before AG-k starts;
  - k'T attention tiles are two full-width [128,8,1024] persistent tiles
    per head (contiguous 2KB lines; a narrow-column variant generated a
    256B-descriptor storm that choked the rings) whose triggers sit right
    after the last v store on the scalar ring, so they fire the moment
    AG-k's semaphore posts - before AG-v begins;
  - wv / wout / bias prefetch triggers are hoisted ahead of dependency-
    stalled compute on their queues so the loads run a full block early;
  - per head, both row-blocks' scores run before any attn@v, which buys
    the v AllGather ~60us of slack.
"""

import os
import sys

import numpy as np

if "/opt/trn_rl_repo" not in sys.path:
    sys.path.insert(0, "/opt/trn_rl_repo")

import ml_dtypes

# ---------------------------------------------------------------- constants
B, S, D = 4, 2048, 2048          # batch, seq, model dim
H, HD = 2, 1024                  # heads, head dim
R = 1024                         # query rows per core
N_CORES = 8
SCALE = 1.0 / 32.0               # HD ** -0.5
NRB = R // 512                   # 512-row blocks

_NC_CACHE = {}
LAST_RESULT = {}


def _build():
    import concourse.bass as bass
    import concourse.tile as tile
    from concourse import bacc, mybir

    F32 = mybir.dt.float32
    F16 = mybir.dt.float16
    BF = mybir.dt.bfloat16
    Exp = mybir.ActivationFunctionType.Exp

    nc = bacc.Bacc("TRN2", target_bir_lowering=False, debug=False,
                   num_devices=N_CORES)

    xT = nc.dram_tensor("xT", [D, R], BF, kind="ExternalInput").ap()
    wqkv = nc.dram_tensor("wqkv", [D, 3 * D], BF, kind="ExternalInput").ap()
    wout = nc.dram_tensor("wout", [D, D], BF, kind="ExternalInput").ap()
    cosk = nc.dram_tensor("cosk", [512, R], F16, kind="ExternalInput").ap()
    sink = nc.dram_tensor("sink", [512, R], F16, kind="ExternalInput").ap()
    bias = nc.dram_tensor("bias", [1, D], F32, kind="ExternalInput").ap()
    out = nc.dram_tensor("out", [R, D], F32, kind="ExternalOutput").ap()

    wq_r = wqkv.rearrange("(c p) m -> p c m", p=128)    # [128, 16, 6144]
    x_r = xT.rearrange("(c p) r -> p c r", p=128)       # [128, 16, R]
    ck_r = cosk.rearrange("(c p) r -> p c r", p=128)    # [128, 4, R]
    sk_r = sink.rearrange("(c p) r -> p c r", p=128)

    def bcast_ap(src_ap, nparts, width):
        return bass.AP(tensor=src_ap.tensor, offset=src_ap.offset,
                       ap=[[0, nparts], [1, width]])

    with tile.TileContext(nc) as tc:
        with (
            tc.tile_pool(name="persist", bufs=1) as persist,
            tc.tile_pool(name="psum", bufs=6, space="PSUM") as psp,
            tc.tile_pool(name="dram", bufs=1, space="DRAM") as dram,
        ):
            # ------------------------------------------- persistent buffers
            qT_sb = persist.tile([128, 16, R], BF, tag="qT")
            aoT_sb = persist.tile([128, 16, R], BF, tag="aoT")
            ones_sb = persist.tile([128, 1], BF, tag="ones")
            nc.vector.memset(ones_sb, 1.0)

            # DRAM scratch
            k_in = dram.tile([D, R], BF, tag="k_in")
            v_in = dram.tile([R, D], BF, tag="v_in")
            k_g = dram.tile([2 * D, R], BF, tag="k_g")     # stacked k'T shards
            v_g = dram.tile([S, D], BF, tag="v_g")         # natural v, global rows

            def load_kth(hi):
                """Attention k'T tiles for head hi: [128, 8, R] per seq-half.

                Persistent pool, scalar ring: the loads fire as soon as the
                AG-k semaphore posts (no pool-open WAR, no sync-ring AG
                starvation)."""
                ts = []
                for sh in range(2):
                    kt = persist.tile([128, 8, R], BF, tag="kTh", bufs=2,
                                      name=f"kth{hi}{sh}")
                    base = sh * D + hi * HD
                    for hf in range(2):
                        nc.scalar.dma_start(
                            out=kt[:, hf * 4:(hf + 1) * 4, :],
                            in_=k_g[base + hf * 512:base + (hf + 1) * 512,
                                    :].rearrange("(c p) r -> p c r", p=128))
                    ts.append(kt)
                return ts

            # =================================================== projections
            with (
                tc.tile_pool(name="proj", bufs=1) as proj,
                tc.tile_pool(name="projs", bufs=1) as projs,
            ):
                # x: interleave column-chunk loads across sync+scalar rings
                x_sb = proj.tile([128, 16, R], BF, tag="xsb")
                for kc in range(16):
                    eng = nc.sync if kc % 2 == 0 else nc.scalar
                    eng.dma_start(out=x_sb[:, kc, :], in_=x_r[:, kc, :])
                # first stationary weights for the k projection
                w_first = []
                for c in (0, 8):
                    wt = projs.tile([128, 16, 128], BF, tag="wst", bufs=5)
                    for hh in (0, 8):
                        nc.gpsimd.dma_start(
                            out=wt[:, hh:hh + 8, :],
                            in_=wq_r[:, hh:hh + 8, D + c * 128:D + (c + 1) * 128])
                    w_first.append(wt)
                ck_sb = proj.tile([128, 4, R], F16, tag="ck")
                sk_sb = proj.tile([128, 4, R], F16, tag="sk")
                nc.gpsimd.dma_start(out=ck_sb, in_=ck_r)
                nc.scalar.dma_start(out=sk_sb, in_=sk_r)

                def load_wst(col0, cc0, dma_eng):
                    wt = projs.tile([128, 16, 128], BF, tag="wst", bufs=5)
                    for hh in (0, 8):
                        dma_eng.dma_start(
                            out=wt[:, hh:hh + 8, :],
                            in_=wq_r[:, hh:hh + 8,
                                     col0 + cc0 * 128:col0 + (cc0 + 1) * 128])
                    return wt

                def load_wv(vc):
                    wv = projs.tile([128, 16, 512], BF, tag="wv", bufs=2,
                                    name=f"wv{vc}")
                    for kc in range(0, 16, 4):
                        nc.scalar.dma_start(
                            out=wv[:, kc:kc + 4, :],
                            in_=wq_r[:, kc:kc + 4,
                                     2 * D + vc * 512:2 * D + (vc + 1) * 512])
                    return wv

                def qk_proj(col0, dest, dma_eng, preloaded=None, post=None):
                    """Project+rope cols [col0, col0+2048) of wqkv.

                    dest(c, rb) -> (apA, apB): bf16 [128,512] destination APs
                    for col-chunk c (head0) and c+8 (head1), row block rb.
                    post(c, rb, apA, apB), if given, runs after the writes."""
                    for c in range(8):
                        if c == 0 and preloaded is not None:
                            w1, w2 = preloaded
                        else:
                            w1 = load_wst(col0, c, dma_eng)
                            w2 = load_wst(col0, c + 8, dma_eng)
                        for rb in range(NRB):
                            rs = slice(rb * 512, (rb + 1) * 512)
                            ps1 = psp.tile([128, 512], F32, tag="mm")
                            ps2 = psp.tile([128, 512], F32, tag="mm")
                            for kc in range(16):
                                nc.tensor.matmul(ps1, w1[:, kc, :], x_sb[:, kc, rs],
                                                 start=kc == 0, stop=kc == 15)
                            for kc in range(16):
                                nc.tensor.matmul(ps2, w2[:, kc, :], x_sb[:, kc, rs],
                                                 start=kc == 0, stop=kc == 15)
                            cosv = ck_sb[:, c % 4, rs]
                            sinv = sk_sb[:, c % 4, rs]
                            t1 = projs.tile([128, 512], F32, tag="rt", bufs=2)
                            t2 = projs.tile([128, 512], F32, tag="rt", bufs=2)
                            apA, apB = dest(c, rb)
                            nc.vector.tensor_mul(t1, ps1, cosv)
                            nc.vector.tensor_mul(t2, ps2, sinv)
                            nc.vector.tensor_sub(apA, t1, t2)
                            nc.vector.tensor_mul(t1, ps2, cosv)
                            nc.vector.tensor_mul(t2, ps1, sinv)
                            nc.vector.tensor_add(apB, t1, t2)
                            if post is not None:
                                post(c, rb, apA, apB)

                # ---- k projection + rope -> k_in, AllGather within pair
                def dest_k(c, rb):
                    oA = projs.tile([128, 512], BF, tag="ro", bufs=3)
                    oB = projs.tile([128, 512], BF, tag="ro", bufs=3)
                    return oA, oB

                def post_k(c, rb, oA, oB):
                    rs = slice(rb * 512, (rb + 1) * 512)
                    nc.gpsimd.dma_start(out=k_in[c * 128:(c + 1) * 128, rs], in_=oA)
                    nc.gpsimd.dma_start(out=k_in[(c + 8) * 128:(c + 9) * 128, rs], in_=oB)

                qk_proj(D, dest_k, nc.scalar, preloaded=w_first, post=post_k)
                nc.gpsimd.collective_compute(
                    "AllGather", bass.mybir.AluOpType.bypass,
                    replica_groups=[[0, 1], [2, 3], [4, 5], [6, 7]],
                    ins=[k_in.opt()], outs=[k_g.opt()])

                # ---- v projection (natural layout). wv prefetch triggers sit
                # at the top of each block, ahead of the dependency-stalled
                # copies, so block vc+1 loads while vc computes
                wv0 = load_wv(0)
                wv_next = None
                for vc in range(4):
                    wv = wv0 if vc == 0 else wv_next
                    if vc + 1 < 4:
                        wv_next = load_wv(vc + 1)
                    for rr in range(R // 128):
                        ps = psp.tile([128, 512], F32, tag="mm")
                        for kc in range(16):
                            nc.tensor.matmul(ps, x_sb[:, kc, rr * 128:(rr + 1) * 128],
                                             wv[:, kc, :], start=kc == 0, stop=kc == 15)
                        vt = projs.tile([128, 512], BF, tag="vo", bufs=3)
                        nc.scalar.copy(vt, ps)
                        nc.scalar.dma_start(
                            out=v_in[rr * 128:(rr + 1) * 128, vc * 512:(vc + 1) * 512],
                            in_=vt)
                nc.gpsimd.collective_compute(
                    "AllGather", bass.mybir.AluOpType.bypass,
                    replica_groups=[[0, 1], [2, 3], [4, 5], [6, 7]],
                    ins=[v_in.opt()], outs=[v_g.opt()])

                # ---- q projection + rope, written directly into qT_sb
                def dest_q(c, rb):
                    rs = slice(rb * 512, (rb + 1) * 512)
                    return qT_sb[:, c, rs], qT_sb[:, c + 8, rs]

                qk_proj(0, dest_q, nc.sync)

                # head-0 attention k'T: triggers fire on AG-k's semaphore,
                # loads land before AG-v starts
                kth0 = load_kth(0)

            # ===================================================== attention
            # fin pool opens first so the wout/bias prefetches below live
            # through the attention phase
            with tc.tile_pool(name="fin", bufs=1) as fin:
                wout_r = wout.rearrange("(c p) m -> p c m", p=128)

                def load_wo(cc):
                    halves = []
                    for hf in range(2):
                        w = fin.tile([128, 8, 512], BF, tag="wo", bufs=3,
                                     name=f"wo{cc}{hf}")
                        for dc in range(0, 8, 2):
                            nc.scalar.dma_start(
                                out=w[:, dc:dc + 2, :],
                                in_=wout_r[:, hf * 8 + dc:hf * 8 + dc + 2,
                                           cc * 512:(cc + 1) * 512])
                        halves.append(w)
                    return halves

                wo0 = load_wo(0)
                bias_sb = fin.tile([128, D], F32, tag="bias")
                nc.scalar.dma_start(out=bias_sb, in_=bcast_ap(bias, 128, D))

                with tc.tile_pool(name="attn", bufs=1) as attn:
                    for hi in range(H):
                        kth = kth0 if hi == 0 else load_kth(hi)
                        v_sb = attn.tile([128, 16, HD], BF, tag="vh")
                        for jc in range(0, 16, 4):
                            nc.gpsimd.dma_start(
                                out=v_sb[:, jc:jc + 4, :],
                                in_=v_g[jc * 128:(jc + 4) * 128,
                                        hi * HD:(hi + 1) * HD].rearrange(
                                    "(c p) m -> p c m", p=128))
                        PTs, recs = [], []
                        # both row blocks' scores first: attn@v then has
                        # ~60us of slack on the v AllGather
                        for rb in range(NRB):
                            rs = slice(rb * 512, (rb + 1) * 512)
                            PT = attn.tile([128, 16, 512], BF, tag="PT", bufs=2)
                            for jc in range(16):
                                kt = kth[jc // 8]
                                j0 = (jc % 8) * 128
                                ps = psp.tile([128, 512], F32, tag="mm")
                                for dc in range(8):
                                    nc.tensor.matmul(
                                        ps, kt[:, dc, j0:j0 + 128],
                                        qT_sb[:, hi * 8 + dc, rs],
                                        start=dc == 0, stop=dc == 7)
                                nc.scalar.activation(PT[:, jc, :], ps, Exp, scale=SCALE)
                            # row sums via ones-matmul, reciprocal, DMA-bcast
                            sps = psp.tile([1, 512], F32, tag="sum", bufs=2)
                            for jc in range(16):
                                nc.tensor.matmul(sps, ones_sb, PT[:, jc, :],
                                                 start=jc == 0, stop=jc == 15)
                            rec = attn.tile([1, 512], F32, tag="rec", bufs=2)
                            nc.vector.reciprocal(rec, sps)
                            rec_d = dram.tile([1, 512], F32, tag="rec_d", bufs=2)
                            nc.scalar.dma_start(out=rec_d, in_=rec)
                            rec_b = attn.tile([128, 512], F32, tag="rec_b", bufs=2)
                            nc.scalar.dma_start(out=rec_b, in_=bcast_ap(rec_d, 128, 512))
                            PTs.append(PT)
                            recs.append(rec_b)
                        for rb in range(NRB):
                            rs = slice(rb * 512, (rb + 1) * 512)
                            PT, rec_b = PTs[rb], recs[rb]
                            for m in range(8):
                                pa = psp.tile([128, 512], F32, tag="mm")
                                for jc in range(16):
                                    nc.tensor.matmul(
                                        pa, v_sb[:, jc, m * 128:(m + 1) * 128],
                                        PT[:, jc, :], start=jc == 0, stop=jc == 15)
                                nc.vector.tensor_mul(aoT_sb[:, hi * 8 + m, rs], pa, rec_b)

                # ============================================ output projection
                for cc in range(4):
                    wo = wo0 if cc == 0 else load_wo(cc)
                    for rr in range(R // 128):
                        r0 = rr * 128
                        ps = psp.tile([128, 512], F32, tag="mm")
                        for dc in range(16):
                            nc.tensor.matmul(ps, aoT_sb[:, dc, r0:r0 + 128],
                                             wo[dc // 8][:, dc % 8, :],
                                             start=dc == 0, stop=dc == 15)
                        ot = fin.tile([128, 512], F32, tag="ot", bufs=2)
                        nc.vector.tensor_add(ot, ps, bias_sb[:, cc * 512:(cc + 1) * 512])
                        nc.gpsimd.dma_start(
                            out=out[r0:r0 + 128, cc * 512:(cc + 1) * 512], in_=ot)

    nc.compile()
    return nc


def _get_nc():
    if "v3" not in _NC_CACHE:
        _NC_CACHE["v3"] = _build()
    return _NC_CACHE["v3"]


def _rope_tables():
    inv_freq = 1.0 / (10000.0 ** (np.arange(0, HD, 2, dtype=np.float32) / HD))
    t = np.arange(S, dtype=np.float32)
    freqs = t[:, None] * inv_freq[None, :]          # (S, 512)
    return np.cos(freqs).astype(np.float32), np.sin(freqs).astype(np.float32)


def kernel(x, wqkv, wout, bout):
    from concourse.bass_utils import run_bass_kernel_spmd

    bf16 = ml_dtypes.bfloat16
    x = np.asarray(x, dtype=np.float32)
    wqkv_b = np.ascontiguousarray(np.asarray(wqkv, dtype=np.float32)).astype(bf16)
    wout_b = np.ascontiguousarray(np.asarray(wout, dtype=np.float32)).astype(bf16)
    bout_f = np.asarray(bout, dtype=np.float32).reshape(1, D)
    cos_h, sin_h = _rope_tables()                   # (S, 512) f32
    cosT = np.ascontiguousarray(cos_h.T)            # (512, S)
    sinT = np.ascontiguousarray(sin_h.T)

    nc = _get_nc()

    in_maps = []
    for c in range(N_CORES):
        bi, half = c // 2, c % 2
        rows = slice(half * R, (half + 1) * R)
        m = {
            "xT": np.ascontiguousarray(x[bi, rows, :].T).astype(bf16),
            "wqkv": wqkv_b,
            "wout": wout_b,
            "bias": bout_f,
            "cosk": np.ascontiguousarray(cosT[:, rows]).astype(np.float16),
            "sink": np.ascontiguousarray(sinT[:, rows]).astype(np.float16),
        }
        in_maps.append(m)

    trace = os.environ.get("KERNEL_TRACE", "0") == "1"
    res = run_bass_kernel_spmd(nc, in_maps, list(range(N_CORES)), trace=trace)
    if trace:
        LAST_RESULT["exec_time_ns"] = res.exec_time_ns
        LAST_RESULT["mean_exec_time_ns"] = res.mean_exec_time_ns

    out_full = np.empty((B, S, D), np.float32)
    for c in range(N_CORES):
        bi, half = c // 2, c % 2
        out_full[bi, half * R:(half + 1) * R, :] = res.results[c]["out"]
    return out_full
